# revision 1
# baseline (speedup 1.0000x reference)
"""ADMM graph-Laplacian block on 8 TRN2 NeuronCores.

Sharding: core = b*H + h  (B=2 x H=4 = 8 fully independent ADMM problems;
the math never mixes b or h until the final comb_weights head-sum, which is
a 4-core AllReduce at the end).

Per-core layout ("compact"): SBUF [128, NG*C] f32, partition p = g*16 + t
(g in 0..7 = GPSIMD core group, t in 0..11 time step, t=12..15 pad).
Free axis = [j in 0..NG) local node, c in 0..4).

kNN gather: DMA compact state -> DRAM stage rows (t-major full-node rows),
DMA stage rows back into a per-group replicated "replica" [128, NPAD*C]
(partition (g,t) holds the FULL node row of time t+shift), then one
gpsimd.ap_gather per event: each Q7 core gathers its group's NG*K edges.

Weighted K-reduce: in-place tensor_tensor mult (weights bf16, c-broadcast
via stride-0 AP) + in-place tree-add over k, then one scalar_tensor_tensor
for the Laplacian combine  out = src*m[p] - agg  (m encodes t boundaries).

CG with fixed input alpha/beta is algebraically collapsed to
x' = x0 + c1*r0 + c2*A(r0), r0 = rhs - A(x0)  (2 operator applies, not 3).
"""
import sys
import os
import numpy as np

sys.path.insert(0, '/opt/trn_rl_repo')

import concourse.bass as bass
import concourse.tile as tile
from concourse import bacc, mybir
from concourse import bass_utils

B, T, N, H, C, K = 2, 12, 2500, 4, 4, 8
AI = 2  # ADMM_ITERS
NPAD = 2560
NGRP = 8
NG = NPAD // NGRP      # 320 nodes per group
NIDX = NG * K          # 2560 gather indices per group
SROW = NPAD * C        # 10240 f32, one full-node time row
N_CORES = 8
F32 = mybir.dt.float32
BF16 = mybir.dt.bfloat16
F16 = mybir.dt.float16
OP = mybir.AluOpType
NSC = 64

SCAL_COLS = {}


def _scol(name):
    if name not in SCAL_COLS:
        SCAL_COLS[name] = len(SCAL_COLS)
        assert len(SCAL_COLS) <= NSC
    return SCAL_COLS[name]


def build(with_cc=True):
    """Build the SPMD Bass program (shared by all 8 cores)."""
    nc = bacc.Bacc("TRN2", target_bir_lowering=False, debug=False,
                   num_devices=N_CORES)
    y0 = nc.declare_dram_parameter("y0", [128, NG * C], F32, isOutput=False)
    wu = nc.declare_dram_parameter("wu", [128, NIDX], F16, isOutput=False)
    wldr = nc.declare_dram_parameter("wldr", [128, NIDX], F16, isOutput=False)
    wldrt = nc.declare_dram_parameter("wldrt", [128, NIDX], F16, isOutput=False)
    idx_in = nc.declare_dram_parameter("idx", [128, NIDX // 16], mybir.dt.int16,
                                       isOutput=False)
    scal_in = nc.declare_dram_parameter("scal", [128, NSC], F32, isOutput=False)
    out_p = nc.declare_dram_parameter("out", [128, NG * C], F32, isOutput=True)

    cc_in = nc.dram_tensor("cc_in", [128, NG * C], F32)
    cc_out = nc.dram_tensor("cc_out", [128, NG * C], F32)

    with tile.TileContext(nc) as tc:
        import contextlib
        ctx = contextlib.ExitStack()
        with ctx:
            state_p = ctx.enter_context(tc.tile_pool(name="state", bufs=1))
            tmp_p = ctx.enter_context(tc.tile_pool(name="tmp", bufs=1))
            mpo_p = ctx.enter_context(tc.tile_pool(name="mpo", bufs=1))
            scr_p = ctx.enter_context(tc.tile_pool(name="scr", bufs=1))
            rep_p = ctx.enter_context(tc.tile_pool(name="rep", bufs=1))
            g_p = ctx.enter_context(tc.tile_pool(name="gat", bufs=2))
            dram_p = ctx.enter_context(tc.tile_pool(name="stage", bufs=2,
                                                    space="DRAM"))
            const_p = ctx.enter_context(tc.tile_pool(name="const", bufs=1))

            # ---- constant loads ------------------------------------------
            w_t = {}
            for nm, par in (("lu", wu), ("ldr", wldr), ("ldrt", wldrt)):
                w = const_p.tile([128, NIDX], F16, tag=f"w_{nm}", name=f"w_{nm}")
                nc.sync.dma_start(w[:], par.ap())
                w_t[nm] = w
            ixs = const_p.tile([128, NIDX // 16], mybir.dt.int16, tag="ixs", name="ixs")
            nc.sync.dma_start(ixs[:], idx_in.ap())
            scal = const_p.tile([128, NSC], F32, tag="scal", name="scal")
            nc.sync.dma_start(scal[:], scal_in.ap())
            zrow = const_p.tile([128, SROW // 128], F16, tag="zrow", name="zrow")
            nc.vector.memset(zrow[:], 0.0)

            def sc(name):
                return scal[:, _scol(name):_scol(name) + 1]

            # ---- states ---------------------------------------------------
            def st(tag):
                return state_p.tile([128, NG * C], F32, tag=tag, name=tag)

            x = st("x")
            zu = st("zu")
            zd = st("zd")
            gam = st("gam")
            gu = st("gu")
            gd = st("gd")
            Hty = st("Hty")
            ldx = st("ldx")

            nc.sync.dma_start(x[:], y0.ap())
            nc.any.tensor_copy(zu[:], x[:])
            nc.any.tensor_copy(zd[:], x[:])
            nc.vector.memset(gam[:], 0.1)
            nc.vector.memset(gu[:], 0.1)
            nc.vector.memset(gd[:], 0.1)
            nc.any.tensor_scalar_mul(Hty[:], x[:], sc("tmask"))

            SHIFT = {"lu": 0, "ldr": -1, "ldrt": +1}

            def mp(src, op, m_name, dst):
                """dst = src*m[p] - sum_k w_op * gather_shift(src)."""
                shift = SHIFT[op]
                srch = scr_p.tile([128, NG * C], F16, tag="s16", name="s16")
                nc.vector.tensor_copy(srch[:], src[:])
                stage = dram_p.tile([128, (18 * SROW) // 128], F16, tag="stage", name="stage")
                flat = stage[:].rearrange("p f -> (p f)")

                def dram_ap(off, dims):
                    return bass.AP(tensor=flat.tensor,
                                   offset=flat.offset + off, ap=dims)

                # zero guard rows 0 and 17 (partition-folding writes)
                nc.sync.dma_start(
                    dram_ap(0, [[SROW // 128, 128], [1, SROW // 128]]),
                    zrow[:])
                nc.sync.dma_start(
                    dram_ap(17 * SROW, [[SROW // 128, 128], [1, SROW // 128]]),
                    zrow[:])
                # compact -> stage rows 1..16 : walk (g, t, i)
                nc.sync.dma_start(
                    dram_ap(SROW, [[NG * C, NGRP], [SROW, 16], [1, NG * C]]),
                    srch[:])
                # stage rows -> replica (per dst group, 12 t rows)
                rep = rep_p.tile([128, SROW], F16, tag="rep", name="rep")
                for gp in range(NGRP):
                    nc.sync.dma_start(
                        rep[gp * 16:(gp + 1) * 16, :],
                        dram_ap((1 + shift) * SROW, [[SROW, 16], [1, SROW]]))
                # gather
                g = g_p.tile([128, NIDX * C], F16, tag="g", name="g")
                nc.gpsimd.ap_gather(g[:], rep[:], ixs[:],
                                    channels=128, num_elems=NPAD, d=C,
                                    num_idxs=NIDX)
                # weighted mult (in place), w broadcast over c
                w = w_t[op]
                w_bc = bass.AP(tensor=w[:].tensor, offset=w[:].offset,
                               ap=[w[:].ap[0], [1, NIDX], [0, C]])
                g3 = g[:].rearrange("p (e c) -> p e c", c=C)
                nc.vector.tensor_tensor(g3, g3, w_bc, op=OP.mult)
                # tree-add over k (in place): view [p, n, k, c]
                g4 = g[:].rearrange("p (n k c) -> p n k c", k=K, c=C)
                nc.vector.tensor_tensor(g4[:, :, 0:4], g4[:, :, 0:4],
                                        g4[:, :, 4:8], op=OP.add)
                nc.vector.tensor_tensor(g4[:, :, 0:2], g4[:, :, 0:2],
                                        g4[:, :, 2:4], op=OP.add)
                nc.vector.tensor_tensor(g4[:, :, 0:1], g4[:, :, 0:1],
                                        g4[:, :, 1:2], op=OP.add)
                agg = g4[:, :, 0, :]  # [p, n, c] strided
                mm = 1.0 if m_name is None else sc(m_name)
                nc.vector.scalar_tensor_tensor(
                    dst[:].rearrange("p (n c) -> p n c", c=C),
                    src[:].rearrange("p (n c) -> p n c", c=C),
                    mm, agg, op0=OP.mult, op1=OP.subtract)

            def tl(pool, tag):
                return pool.tile([128, NG * C], F32, tag=tag, name=tag)

            def stt(out, in0, s, in1, op0=OP.mult, op1=OP.add):
                nc.vector.scalar_tensor_tensor(out[:], in0[:], s, in1[:],
                                               op0=op0, op1=op1)

            # ---- E0: ldx = Ldr(x0)  (= phi0) ------------------------------
            mp(x, "ldr", "m_ldr", ldx)

            tmp_next = None
            for i in range(AI):
                if i == 0:
                    tmp = tl(tmp_p, "tmp_rhs")
                    stt(tmp, ldx, sc(f"rho{i}"), gam)
                else:
                    tmp = tmp_next
                # E_a: Ldrt(tmp) and rhs chain
                Ltmp = tl(mpo_p, "Ltmp")
                mp(tmp, "ldrt", "m_ldrt", Ltmp)
                a1 = tl(scr_p, "s1")
                stt(a1, zu, sc(f"rhou2{i}"), Hty)
                a2 = tl(scr_p, "s2")
                stt(a2, zd, sc(f"rhod2{i}"), a1)
                a3 = tl(scr_p, "s1")
                stt(a3, gu, -0.5, a2)
                a4 = tl(scr_p, "s2")
                stt(a4, gd, -0.5, a3)
                rhs = tl(tmp_p, "rhs")
                stt(rhs, Ltmp, 0.5, a4)
                # E_b: cldr(x0) part 2
                cx = tl(mpo_p, "cx")
                mp(ldx, "ldrt", "m_ldrt", cx)
                t1 = tl(scr_p, "s1")
                nc.any.tensor_scalar_mul(t1[:], x[:], sc(f"mA{i}"))
                Ax0 = tl(scr_p, "s2")
                stt(Ax0, cx, sc(f"rhoh{i}"), t1)
                r0 = tl(tmp_p, "r0")
                nc.vector.tensor_tensor(r0[:], rhs[:], Ax0[:], op=OP.subtract)
                # E_c, E_d: A(r0); x update
                w2 = tl(mpo_p, "w2")
                mp(r0, "ldr", "m_ldr", w2)
                cr0 = tl(mpo_p, "cr0")
                mp(w2, "ldrt", "m_ldrt", cr0)
                t2 = tl(scr_p, "s1")
                nc.any.tensor_scalar_mul(t2[:], r0[:], sc(f"mA{i}"))
                Ar0 = tl(scr_p, "s2")
                stt(Ar0, cr0, sc(f"rhoh{i}"), t2)
                stt(x, r0, sc(f"c1x{i}"), x)
                stt(x, Ar0, sc(f"c2x{i}"), x)

                # E_e: zu apply 1
                Lzu = tl(mpo_p, "Ltmp")
                mp(zu, "lu", None, Lzu)
                tu = tl(scr_p, "s1")
                nc.any.tensor_scalar_mul(tu[:], zu[:], sc(f"rhou2{i}"))
                Azu = tl(scr_p, "s2")
                stt(Azu, Lzu, sc(f"muu{i}"), tu)
                ru_ = tl(scr_p, "s1")
                nc.any.tensor_scalar_mul(ru_[:], x[:], sc(f"rhou2{i}"))
                rhsu = tl(scr_p, "s3")
                stt(rhsu, gu, 0.5, ru_)
                r0u = tl(tmp_p, "r0u")
                nc.vector.tensor_tensor(r0u[:], rhsu[:], Azu[:], op=OP.subtract)

                # E_f: zd apply 1a ; E_g: ldx = Ldr(x_new) ; E_h: zd apply 1b
                w1d = tl(mpo_p, "w2")
                mp(zd, "ldr", "m_ldr", w1d)
                mp(x, "ldr", "m_ldr", ldx)
                czd = tl(mpo_p, "cx")
                mp(w1d, "ldrt", "m_ldrt", czd)
                td = tl(scr_p, "s1")
                nc.any.tensor_scalar_mul(td[:], zd[:], sc(f"rhod2{i}"))
                Azd = tl(scr_p, "s2")
                stt(Azd, czd, sc(f"mud2{i}"), td)
                rd_ = tl(scr_p, "s1")
                nc.any.tensor_scalar_mul(rd_[:], x[:], sc(f"rhod2{i}"))
                rhsd = tl(scr_p, "s3")
                stt(rhsd, gd, 0.5, rd_)
                r0d = tl(tmp_p, "r0d")
                nc.vector.tensor_tensor(r0d[:], rhsd[:], Azd[:], op=OP.subtract)

                # E_i: zu apply 2 -> zu update
                Lru = tl(mpo_p, "Ltmp")
                mp(r0u, "lu", None, Lru)
                tu2 = tl(scr_p, "s1")
                nc.any.tensor_scalar_mul(tu2[:], r0u[:], sc(f"rhou2{i}"))
                Aru = tl(scr_p, "s2")
                stt(Aru, Lru, sc(f"muu{i}"), tu2)
                stt(zu, r0u, sc(f"c1u{i}"), zu)
                stt(zu, Aru, sc(f"c2u{i}"), zu)

                # E_j, E_k: zd apply 2 -> zd update
                w2d = tl(mpo_p, "w2")
                mp(r0d, "ldr", "m_ldr", w2d)
                crd = tl(mpo_p, "cx")
                mp(w2d, "ldrt", "m_ldrt", crd)
                td2 = tl(scr_p, "s1")
                nc.any.tensor_scalar_mul(td2[:], r0d[:], sc(f"rhod2{i}"))
                Ard = tl(scr_p, "s2")
                stt(Ard, crd, sc(f"mud2{i}"), td2)
                stt(zd, r0d, sc(f"c1d{i}"), zd)
                stt(zd, Ard, sc(f"c2d{i}"), zd)

                # duals
                du = tl(scr_p, "s1")
                stt(du, x, sc(f"rhou{i}"), gu)
                stt(gu, zu, sc(f"nrhou{i}"), du)
                dd = tl(scr_p, "s1")
                stt(dd, x, sc(f"rhod{i}"), gd)
                stt(gd, zd, sc(f"nrhod{i}"), dd)

                # phi block (uses ldx = Ldr(x_new))
                s_ = tl(scr_p, "s1")
                stt(s_, gam, sc(f"ninvrho{i}"), ldx)
                pa = tl(scr_p, "s2")
                nc.vector.tensor_scalar(pa[:], s_[:], sc(f"nlam{i}"), 0.0,
                                        op0=OP.add, op1=OP.max)
                pb = tl(scr_p, "s3")
                nc.vector.tensor_scalar(pb[:], s_[:], sc(f"lam{i}"), 0.0,
                                        op0=OP.add, op1=OP.min)
                phi = tl(tmp_p, "phi")
                nc.vector.tensor_tensor(phi[:], pa[:], pb[:], op=OP.add)
                tg = tl(scr_p, "s2")
                stt(tg, phi, sc(f"rho{i}"), gam)
                stt(gam, ldx, sc(f"nrho{i}"), tg)
                if i + 1 < AI:
                    tmp_next = tl(tmp_p, "tmp_rhs")
                    stt(tmp_next, phi, sc(f"rho{i+1}"), gam)

            # ---- output: oc = x*cw ; AllReduce over 4-core groups ---------
            oc = tl(tmp_p, "oc")
            nc.any.tensor_scalar_mul(oc[:], x[:], sc("cw"))
            if not with_cc:
                nc.sync.dma_start(out_p.ap(), oc[:])
            ccsem = nc.alloc_semaphore("ccsem")
            dsem = nc.alloc_semaphore("dsem")
            if with_cc:
              with tc.tile_critical():
                  nc.gpsimd.dma_start(cc_in.ap(), oc[:]).then_inc(dsem, 16)
                  nc.gpsimd.nop()._wait_ge(dsem, 16)
                  nc.gpsimd.collective_compute(
                      "AllReduce", OP.add,
                      replica_groups=[[0, 1, 2, 3], [4, 5, 6, 7]],
                      ins=[cc_in.ap().opt()],
                      outs=[cc_out.ap().opt()]).then_inc(ccsem, 1)
                  nc.gpsimd.nop()._wait_ge(ccsem, 1)
                  nc.gpsimd.dma_start(out_p.ap(), cc_out.ap()).then_inc(dsem, 16)
                  nc.gpsimd.nop()._wait_ge(dsem, 32)
    nc.compile()
    return nc


# =======================================================================
# host-side data prep
# =======================================================================

def _compact(arr_tnc):
    """(T, NPAD, C) -> [128, NG*C] compact layout."""
    out = np.zeros((128, NG * C), np.float32)
    a = np.asarray(arr_tnc, np.float32).reshape(T, NGRP, NG * C)
    for g in range(NGRP):
        out[g * 16:g * 16 + T] = a[:, g]
    return out


def _uncompact(dev):
    """[128, NG*C] -> (T, N, C)."""
    out = np.zeros((T, NPAD, C), np.float32)
    for g in range(NGRP):
        out[:, g * NG:(g + 1) * NG] = dev[g * 16:g * 16 + T].reshape(
            T, NG, C)
    return out[:, :N]


def prep_inputs(y, u_ew, d_ew, mu_u, mu_d1, mu_d2, rho, rho_u, rho_d,
                alpha_x, beta_x, alpha_zu, beta_zu, alpha_zd, beta_zd,
                comb_weights, knn, mask):
    import ml_dtypes
    y = np.asarray(y, np.float32)
    knn = np.asarray(knn).astype(np.int64)
    mask = int(np.asarray(mask))

    idx_np = np.zeros((128, NIDX // 16), np.int16)
    knn_pad = np.zeros((NPAD, K), np.int16)
    knn_pad[:N] = knn.astype(np.int16)
    for g in range(NGRP):
        lst = knn_pad[g * NG:(g + 1) * NG].reshape(-1)  # n-major, k-minor
        idx_np[g * 16:(g + 1) * 16, :] = lst.reshape(NIDX // 16, 16).T

    u_ew = np.asarray(u_ew, np.float32)   # (N, T, K, H)
    d_ew = np.asarray(d_ew, np.float32)   # (N, T-1, K, H)

    def wt(h, kind):
        w = np.zeros((128, NIDX), np.float32)
        for g in range(NGRP):
            n0 = g * NG
            n1r = min(n0 + NG, N)
            cnt = n1r - n0
            if cnt <= 0:
                continue
            for t in range(T):
                if kind == "lu":
                    src = u_ew[n0:n1r, t, :, h]
                elif kind == "ldr":
                    if t == 0:
                        continue
                    src = d_ew[n0:n1r, t - 1, :, h]
                else:  # ldrt
                    if t >= T - 1:
                        continue
                    src = d_ew[n0:n1r, t, :, h]
                w[g * 16 + t, :cnt * K] = src.reshape(-1)
        return w.astype(np.float16)

    def pvec(fn):
        v = np.zeros((128, 1), np.float32)
        for g in range(NGRP):
            for t in range(T):
                v[g * 16 + t, 0] = fn(t)
        return v

    tmask_v = pvec(lambda t: 1.0 if t < mask else 0.0)
    m_ldr_v = pvec(lambda t: 1.0 if t >= 1 else 0.0)
    m_ldrt_v = pvec(lambda t: 1.0 if 1 <= t <= T - 2 else 0.0)

    def cg_coef(al, be, i, h):
        a0 = float(al[i, 0, h, 0]); a1 = float(al[i, 1, h, 0])
        b0 = float(be[i, 0, h, 0])
        return a0 + a1 * (1.0 + b0), -a0 * a1

    in_maps = []
    for core in range(N_CORES):
        b, h = core // H, core % H
        ypad = np.zeros((T, NPAD, C), np.float32)
        ypad[:, :N] = y[b]
        m = {
            "y0": _compact(ypad),
            "wu": wt(h, "lu"),
            "wldr": wt(h, "ldr"),
            "wldrt": wt(h, "ldrt"),
            "idx": idx_np,
        }
        scal = np.zeros((128, NSC), np.float32)

        def put(name, val):
            scal[:, _scol(name)] = val

        def putv(name, vec):
            scal[:, _scol(name):_scol(name) + 1] = vec

        putv("tmask", tmask_v)
        putv("m_ldr", m_ldr_v)
        putv("m_ldrt", m_ldrt_v)
        put("cw", float(np.asarray(comb_weights)[h]))
        for i in range(AI):
            rho_i = float(np.asarray(rho)[i])
            rhou_i = float(np.asarray(rho_u)[i])
            rhod_i = float(np.asarray(rho_d)[i])
            put(f"rho{i}", rho_i)
            put(f"nrho{i}", -rho_i)
            put(f"ninvrho{i}", -1.0 / rho_i)
            put(f"rhoh{i}", rho_i / 2)
            put(f"rhou{i}", rhou_i)
            put(f"nrhou{i}", -rhou_i)
            put(f"rhou2{i}", rhou_i / 2)
            put(f"rhod{i}", rhod_i)
            put(f"nrhod{i}", -rhod_i)
            put(f"rhod2{i}", rhod_i / 2)
            put(f"muu{i}", float(np.asarray(mu_u)[i]))
            put(f"mud2{i}", float(np.asarray(mu_d2)[i]))
            lam = float(np.asarray(mu_d1)[i]) / rho_i
            put(f"lam{i}", lam)
            put(f"nlam{i}", -lam)
            putv(f"mA{i}", tmask_v + (rhou_i + rhod_i) / 2)
            c1, c2 = cg_coef(np.asarray(alpha_x), np.asarray(beta_x), i, h)
            put(f"c1x{i}", c1); put(f"c2x{i}", c2)
            c1, c2 = cg_coef(np.asarray(alpha_zu), np.asarray(beta_zu), i, h)
            put(f"c1u{i}", c1); put(f"c2u{i}", c2)
            c1, c2 = cg_coef(np.asarray(alpha_zd), np.asarray(beta_zd), i, h)
            put(f"c1d{i}", c1); put(f"c2d{i}", c2)
        m["scal"] = scal
        in_maps.append(m)
    return in_maps


_NC_CACHE = {}


def get_nc():
    if "nc" not in _NC_CACHE:
        _NC_CACHE["nc"] = build()
    return _NC_CACHE["nc"]


def kernel(**inputs):
    nc = get_nc()
    in_maps = prep_inputs(**inputs)
    res = bass_utils.run_bass_kernel_spmd(nc, in_maps,
                                          core_ids=list(range(N_CORES)))
    out = np.zeros((B, T, N, C), np.float32)
    for b in range(B):
        out[b] = _uncompact(res.results[b * H]["out"])
    return out

